# revision 13
# baseline (speedup 1.0000x reference)
"""Trainium2 Bass kernel for nn_CNF1D: 1-D continuous normalizing flow.

Reference computation (per sample b, D=1, H=256, RK4 with 4 steps over [0,1]):
    f(t,z):  h1 = tanh(z*W1[0] + t*W1[1] + b1); h2 = tanh(h1@W2 + b2);
             f = h2@W3 + b3
    JVP:     s1 = 1-h1^2;  g2 = (1-h2^2) * ((s1*W1[0])@W2);  df = g2@W3
    (z, div) integrated with RK4; outputs (z_final, div_integral).

Strategy: pure data parallelism over 8 cores (4096 samples each), organized
as 4 PAIRS of 512-sample chunks per core (processed in 2 quads of 2 pairs).
Hidden-major layout ([hidden, batch]); hidden lives on SBUF partitions.

Per-pair state tile U [128, 512] (fp32r):
  p0-5:    c0 [z, k1z..k4z, ones]   p6-10:  c0 [div, k1d..k4d]
  p32-37:  c0 replica of p0-5 (for row-tiled input matmuls)
  p64-74:  c1 (as p0-10)            p96-101: c1 replica
Input layer: 4 concurrent row-tiled K=6 matmuls (row groups 0/32/64/96).
Output layer: col-tiled M<=2 matmuls (f duplicated to 2 partitions so the
k-row gather including the replica is a single strided DMA).
Evacuations: layer tanh via wide ACTs ([128,1024]); g2 = (g2p + C2) * s2 via
one STT per mo-half over the whole pair ([128,1024]); collector copied once
per QUAD ([128,1024]).

dtypes: state + input/combine matmuls fp32r; streams bf16; PSUM fp32.
"""

import sys

for _p in ("/opt/trn_rl_repo",):
    if _p not in sys.path:
        sys.path.insert(0, _p)

import numpy as np
import ml_dtypes

import concourse.mybir as mybir
from concourse import bacc, tile
from concourse.bass_utils import run_bass_kernel_spmd

F32 = mybir.dt.float32
F32R = mybir.dt.float32r
BF16 = mybir.dt.bfloat16
ALU = mybir.AluOpType
TANH = mybir.ActivationFunctionType.Tanh
COPY = mybir.ActivationFunctionType.Copy

N_CORES = 8
B_TOT = 32768
B = B_TOT // N_CORES        # 4096 per core
H = 256                     # hidden
CH = 512                    # chunk (matmul N / psum bank)
NCH = B // CH               # 8 chunks per core
NPAIR = NCH // 2            # 4 pairs
N_STEPS = 4
DT = 1.0 / N_STEPS
N_EVALS = 4 * N_STEPS       # 16
STAGE_OFF = [0.0, DT / 2, DT / 2, DT]
STAGE_C = [0.0, DT / 2, DT / 2, DT]


def _f32r(x):
    """Round to fp32r (11 explicit mantissa bits, RNE)."""
    b = np.ascontiguousarray(np.asarray(x, np.float32)).view(np.uint32)
    r = (b + np.uint32(0x7FF) + ((b >> np.uint32(12)) & np.uint32(1))) & np.uint32(
        0xFFFFF000
    )
    return r.view(np.float32).copy()


def _build_nc():
    nc = bacc.Bacc("TRN2", target_bir_lowering=False, debug=False,
                   num_devices=N_CORES)

    t0u = nc.dram_tensor("t0u", (NPAIR, 128, CH), F32R, kind="ExternalInput")
    # input-layer weights, replicated per row group:
    #  p0-5: m0-half, p32-37: m1-half, p64-69: m0, p96-101: m1
    lin = nc.dram_tensor("lin", (128, N_EVALS * 128), F32R, kind="ExternalInput")
    combzd = nc.dram_tensor("combzd", (128, 3), F32R, kind="ExternalInput")
    w2 = nc.dram_tensor("w2", (128, 512), BF16, kind="ExternalInput")
    w2gn = nc.dram_tensor("w2gn", (128, 512), BF16, kind="ExternalInput")
    # w3t cols: per mo-half: [f, f, df]  (f duplicated for the M=2 output)
    w3 = nc.dram_tensor("w3", (128, 6), BF16, kind="ExternalInput")
    c2 = nc.dram_tensor("c2", (128, 2), F32, kind="ExternalInput")
    b2 = nc.dram_tensor("b2", (128, 2), F32, kind="ExternalInput")

    zf = nc.dram_tensor("zf", (NCH, CH), F32R, kind="ExternalOutput")
    dv = nc.dram_tensor("dv", (NCH, CH), F32R, kind="ExternalOutput")

    with tile.TileContext(nc) as tc:
        with (
            tc.tile_pool(name="const", bufs=1) as cpool,
            tc.tile_pool(name="state", bufs=1) as spool,
            tc.tile_pool(name="work", bufs=5) as wpool,
            tc.tile_pool(name="pmain", bufs=2, space="PSUM") as pmain,
            tc.tile_pool(name="pcoll", bufs=2, space="PSUM") as pcoll,
        ):
            lint = cpool.tile([128, N_EVALS * 128], F32R)
            combt = cpool.tile([128, 3], F32R)
            w2t = cpool.tile([128, 512], BF16)
            w2gnt = cpool.tile([128, 512], BF16)
            w3t = cpool.tile([128, 6], BF16)
            c2t = cpool.tile([128, 2], F32)
            b2t = cpool.tile([128, 2], F32)
            nc.sync.dma_start(lint[:], lin[:])
            nc.sync.dma_start(combt[:], combzd[:])
            nc.sync.dma_start(w2t[:], w2[:])
            nc.sync.dma_start(w2gnt[:], w2gn[:])
            nc.sync.dma_start(w3t[:], w3[:])
            nc.sync.dma_start(c2t[:], c2[:])
            nc.sync.dma_start(b2t[:], b2[:])

            U = []
            for p in range(NPAIR):
                u = spool.tile([128, CH], F32R, tag=f"U{p}")
                nc.sync.dma_start(u[:], t0u[p, :, :])
                U.append(u)

            # round-robin DMA queues for gathers
            dmaq = [nc.sync, nc.gpsimd]
            qi = [0]

            def gdma(dst, src):
                dmaq[qi[0] % 2].dma_start(dst, src)
                qi[0] += 1

            for e in range(N_EVALS):
                s = e % 4
                lslice = lint[:, e * 128:(e + 1) * 128]
                for q in range(2):
                    # -------- per-quad processing (pairs 2q, 2q+1) --------
                    pairs = [2 * q, 2 * q + 1]
                    h2b = {}
                    g2b = {}
                    for pp, p in enumerate(pairs):
                        Up = U[p]
                        # ---- layer 1: 4 row-tiled K=6 matmuls ----
                        h1b = wpool.tile([128, 2048], BF16, tag="h1")
                        pre0 = pmain.tile([128, 1024], F32, tag="a2")
                        pre1 = pmain.tile([128, 1024], F32, tag="a2")
                        pres = [pre0, pre1]
                        for c in range(2):  # chunk within pair
                            for m in range(2):  # m-half of hidden1
                                rg = 64 * c + 32 * m
                                nc.tensor.matmul(
                                    pres[c][:, m * CH:(m + 1) * CH],
                                    lint[rg:rg + 6, e * 128:(e + 1) * 128],
                                    Up[rg:rg + 6, :],
                                    tile_position=(rg, 0),
                                )
                        for c in range(2):
                            nc.scalar.activation(
                                h1b[:, c * 1024:(c + 1) * 1024], pres[c][:],
                                TANH
                            )
                        sq1 = wpool.tile([128, 2048], BF16, tag="sq1")
                        nc.vector.tensor_tensor(sq1[:], h1b[:], h1b[:], ALU.mult)

                        # ---- layer 2 ----
                        h2 = wpool.tile([128, 2048], BF16, tag="h2")
                        s2 = wpool.tile([128, 2048], BF16, tag="s2")
                        g2 = wpool.tile([128, 2048], BF16, tag="g2")
                        for mo in range(2):
                            mslice = slice(mo * 1024, (mo + 1) * 1024)
                            a2 = pmain.tile([128, 1024], F32, tag="a2")
                            for k in range(2):
                                for c in range(2):
                                    nc.tensor.matmul(
                                        a2[:, c * CH:(c + 1) * CH],
                                        w2t[:, k * 256 + mo * 128:
                                            k * 256 + (mo + 1) * 128],
                                        h1b[:, c * 1024 + k * CH:
                                            c * 1024 + (k + 1) * CH],
                                        start=(k == 0),
                                        stop=(k == 1),
                                    )
                            nc.scalar.activation(
                                h2[:, mslice], a2[:], TANH,
                                bias=b2t[:, mo:mo + 1],
                            )
                            # s2 = 1 - h2^2 (per mo-half, so the g-stream STT
                            # for this half never waits on the other half)
                            sq2 = wpool.tile([128, 1024], BF16, tag="sq2")
                            nc.vector.tensor_tensor(
                                sq2[:], h2[:, mslice], h2[:, mslice], ALU.mult
                            )
                            nc.vector.tensor_scalar(
                                s2[:, mslice], sq2[:], -1.0, 1.0,
                                ALU.mult, ALU.add,
                            )
                            g2p = pmain.tile([128, 1024], F32, tag="a2")
                            for k in range(2):
                                for c in range(2):
                                    nc.tensor.matmul(
                                        g2p[:, c * CH:(c + 1) * CH],
                                        w2gnt[:, k * 256 + mo * 128:
                                              k * 256 + (mo + 1) * 128],
                                        sq1[:, c * 1024 + k * CH:
                                            c * 1024 + (k + 1) * CH],
                                        start=(k == 0),
                                        stop=(k == 1),
                                    )
                            nc.vector.scalar_tensor_tensor(
                                g2[:, mslice], g2p[:],
                                c2t[:, mo:mo + 1],
                                s2[:, mslice],
                                ALU.add, ALU.mult,
                            )
                        h2b[pp] = h2
                        g2b[pp] = g2

                    # ---- output layer for the quad: col-tiled into coll ----
                    coll = pcoll.tile([128, 1024], F32, tag="coll")
                    for pp in range(2):
                        off = pp * CH
                        for mo in range(2):
                            st, sp = (mo == 0), (mo == 1)
                            for c in range(2):
                                # f -> partitions {64c, 64c+1} (M=2 dup)
                                nc.tensor.matmul(
                                    coll[64 * c:64 * c + 2, off:off + CH],
                                    w3t[:, 3 * mo:3 * mo + 2],
                                    h2b[pp][:, mo * 1024 + c * CH:
                                            mo * 1024 + (c + 1) * CH],
                                    start=st, stop=sp,
                                    tile_position=(0, 64 * c),
                                )
                                # df -> partition {64c+32}
                                nc.tensor.matmul(
                                    coll[64 * c + 32:64 * c + 33, off:off + CH],
                                    w3t[:, 3 * mo + 2:3 * mo + 3],
                                    g2b[pp][:, mo * 1024 + c * CH:
                                            mo * 1024 + (c + 1) * CH],
                                    start=st, stop=sp,
                                    tile_position=(0, 64 * c + 32),
                                )
                    scr = wpool.tile([128, 1024], F32R, tag="scr")
                    nc.scalar.activation(scr[:], coll[:], COPY)

                    # ---- gathers: k-rows back into U (incl. replicas) ----
                    for pp, p in enumerate(pairs):
                        off = pp * CH
                        Up = U[p]
                        for c in range(2):
                            base = 64 * c
                            # f (2 copies) -> z-block k-row + replica
                            gdma(Up[base + 1 + s:base + 34 + s:32, :],
                                 scr[base:base + 2, off:off + CH])
                            # df -> div-block k-row
                            gdma(Up[base + 7 + s:base + 8 + s, :],
                                 scr[base + 32:base + 33, off:off + CH])

                    if s == 3:
                        # ---- RK4 combine: one K=11 M=3 matmul per chunk.
                        # fp32r blocks col-tiling, so every matmul writes
                        # rows 0-2 (col group 0); chunks get separate
                        # 512-col halves of a per-pair cc tile. ----
                        for pp, p in enumerate(pairs):
                            cc = pmain.tile([128, 1024], F32, tag="a2")
                            for c in range(2):
                                base = 64 * c
                                nc.tensor.matmul(
                                    cc[0:3, c * CH:(c + 1) * CH],
                                    combt[base:base + 11, :],
                                    U[p][base:base + 11, :],
                                    tile_position=(base, 0),
                                )
                            scr2 = wpool.tile([128, 1024], F32R, tag="scr")
                            nc.scalar.activation(
                                scr2[0:3, :], cc[0:3, :], COPY)
                            for c in range(2):
                                base = 64 * c
                                ccol = c * CH
                                ch = 4 * q + 2 * pp + c
                                if e == N_EVALS - 1:
                                    nc.sync.dma_start(
                                        zf[ch:ch + 1, :],
                                        scr2[0:1, ccol:ccol + CH])
                                    nc.sync.dma_start(
                                        dv[ch:ch + 1, :],
                                        scr2[2:3, ccol:ccol + CH])
                                else:
                                    # z -> rows {0, 32}+base, div -> row 6+base
                                    gdma(U[p][base:base + 33:32, :],
                                         scr2[0:2, ccol:ccol + CH])
                                    gdma(U[p][base + 6:base + 7, :],
                                         scr2[2:3, ccol:ccol + CH])

    nc.compile()
    return nc


_NC_CACHE = None


def _get_nc():
    global _NC_CACHE
    if _NC_CACHE is None:
        _NC_CACHE = _build_nc()
    return _NC_CACHE


def _host_prep(z0, W1, b1, W2, b2, W3, b3):
    """Build per-core input maps (host-side folds; all tiny)."""
    z0 = np.asarray(z0, np.float32)
    W1 = np.asarray(W1, np.float32)
    b1 = np.asarray(b1, np.float32)
    W2 = np.asarray(W2, np.float32)
    b2v = np.asarray(b2, np.float32)
    W3 = np.asarray(W3, np.float32)
    b3v = float(np.asarray(b3, np.float32).reshape(()))

    w1r0, w1r1 = W1[0], W1[1]

    # lin: [128 partitions, N_EVALS*128]; row groups 0/64 carry the m0-half
    # of the input weights, 32/96 the m1-half (6 K-rows each).
    lin = np.zeros((128, N_EVALS * 128), np.float32)
    for e in range(N_EVALS):
        i, s = divmod(e, 4)
        t_e = i * DT + STAGE_OFF[s]
        c_e = STAGE_C[s]
        blk6 = np.zeros((6, H), np.float32)
        blk6[0] = w1r0
        if s >= 1:
            blk6[s] = c_e * w1r0
        blk6[5] = t_e * w1r1 + b1 + c_e * b3v * w1r0
        for rg, m in ((0, 0), (32, 1), (64, 0), (96, 1)):
            lin[rg:rg + 6, e * 128:(e + 1) * 128] = blk6[:, m * 128:(m + 1) * 128]

    combzd = np.zeros((128, 3), np.float32)
    zcol = [1.0, DT / 6, DT / 3, DT / 3, DT / 6, DT * b3v, 0, 0, 0, 0, 0]
    dcol = [0, 0, 0, 0, 0, 0, 1.0, DT / 6, DT / 3, DT / 3, DT / 6]
    for base in (0, 64):
        combzd[base:base + 11, 0] = zcol
        combzd[base:base + 11, 1] = zcol
        combzd[base:base + 11, 2] = dcol

    w2p = np.concatenate([W2[0:128, :], W2[128:256, :]], axis=1)  # [128,512]
    w2g = W2 * w1r0[:, None]
    w2gnp = np.concatenate([-w2g[0:128, :], -w2g[128:256, :]], axis=1)
    c2 = w2g.sum(axis=0)  # [256]
    c2p = np.stack([c2[0:128], c2[128:256]], axis=1)  # [128,2]
    b2p = np.stack([b2v[0:128], b2v[128:256]], axis=1)
    # w3t: per mo-half cols [f, f, df]
    w3p = np.zeros((128, 6), np.float32)
    for mo in range(2):
        col = W3[mo * 128:(mo + 1) * 128, 0]
        w3p[:, 3 * mo] = col
        w3p[:, 3 * mo + 1] = col
        w3p[:, 3 * mo + 2] = col

    shared = {
        "lin": _f32r(lin),
        "combzd": _f32r(combzd),
        "w2": w2p.astype(ml_dtypes.bfloat16),
        "w2gn": w2gnp.astype(ml_dtypes.bfloat16),
        "w3": w3p.astype(ml_dtypes.bfloat16),
        "c2": c2p,
        "b2": b2p,
    }
    in_maps = []
    for core in range(N_CORES):
        zc = z0[core * B:(core + 1) * B, 0].reshape(NCH, CH)
        t0uv = np.zeros((NPAIR, 128, CH), np.float32)
        for p in range(NPAIR):
            for c in range(2):
                base = 64 * c
                zrow = _f32r(zc[2 * p + c])
                t0uv[p, base + 0, :] = zrow
                t0uv[p, base + 5, :] = 1.0
                t0uv[p, base + 32, :] = zrow
                t0uv[p, base + 37, :] = 1.0
        in_maps.append({"t0u": t0uv, **shared})
    return in_maps


def _run(in_maps, **kw):
    nc = _get_nc()
    return run_bass_kernel_spmd(nc, in_maps, core_ids=list(range(N_CORES)), **kw)


def kernel(z0, W1, b1, W2, b2, W3, b3):
    in_maps = _host_prep(z0, W1, b1, W2, b2, W3, b3)
    res = _run(in_maps)
    zf = np.concatenate(
        [np.asarray(r["zf"], np.float32).reshape(B, 1) for r in res.results]
    )
    dv = np.concatenate(
        [np.asarray(r["dv"], np.float32).reshape(B, 1) for r in res.results]
    )
    return zf, dv


# revision 16
# speedup vs baseline: 1.8247x; 1.8247x over previous
"""Trainium2 Bass kernel for nn_CNF1D: 1-D continuous normalizing flow.

Reference computation (per sample b, D=1, H=256, RK4 with 4 steps over [0,1]):
    f(t,z):  h1 = tanh(z*W1[0] + t*W1[1] + b1); h2 = tanh(h1@W2 + b2);
             f = h2@W3 + b3
    JVP:     s1 = 1-h1^2;  g2 = (1-h2^2) * ((s1*W1[0])@W2);  df = g2@W3
    (z, div) integrated with RK4; outputs (z_final, div_integral).

Strategy: pure data parallelism over 8 cores (4096 samples each), as 4 PAIRS
of 512-sample chunks. Hidden-major layout (hidden on SBUF partitions).

KEY TRICK - no per-step RK4 combine: every stage state z_s is LINEAR in
(z0, k-history), so the K=18 input matmul reconstructs z_s directly from the
stored k-rows with host-folded weights. k/kd values accumulate in U across
all 16 evals; one final matmul pair produces (zf, div).

Per-pair state tile U [128, 512] fp32r; chunk c block at B = 64*c:
  B+0: z0   B+1: ones   B+2..17: k1..k4 for steps 0..3 (z-side)
  B+18..25: kd1..kd4 steps 0,1
  B+32..49: replica of B+0..17 (for the 2nd row-tiled input matmul)
  B+50..57: kd1..kd4 steps 2,3
Input layer: 4 concurrent row-tiled K=18 matmuls (row groups 0/32/64/96).
Output layer: col-tiled bf16 matmuls; f duplicated to 2 partitions (M=2) so
the k-row gather (main + replica) is one stride-32 DMA.
Evacuations: wide ACTs ([128,1024]); g2 = (g2p + C2)*s2 as one STT per
mo-half over the whole pair ([128,1024] from PSUM).

dtypes: state + input/final matmuls fp32r; streams bf16; PSUM fp32.
"""

import sys

for _p in ("/opt/trn_rl_repo",):
    if _p not in sys.path:
        sys.path.insert(0, _p)

import numpy as np
import ml_dtypes

import concourse.mybir as mybir
from concourse import bacc, tile
from concourse.bass_utils import run_bass_kernel_spmd

F32 = mybir.dt.float32
F32R = mybir.dt.float32r
BF16 = mybir.dt.bfloat16
ALU = mybir.AluOpType
TANH = mybir.ActivationFunctionType.Tanh
COPY = mybir.ActivationFunctionType.Copy

N_CORES = 8
B_TOT = 32768
B = B_TOT // N_CORES        # 4096 per core
H = 256                     # hidden
CH = 512                    # chunk (matmul N / psum bank)
NCH = B // CH               # 8 chunks per core
NPAIR = NCH // 2            # 4 pairs
N_STEPS = 4
DT = 1.0 / N_STEPS
N_EVALS = 4 * N_STEPS       # 16
STAGE_OFF = [0.0, DT / 2, DT / 2, DT]
STAGE_C = [0.0, DT / 2, DT / 2, DT]
RK4W = [DT / 6, DT / 3, DT / 3, DT / 6]
KIN = 18                    # input contraction rows: z, ones, 16 k-rows


def _kd_row(i, s):
    """U row offset (within a chunk block) of kd for step i, stage s."""
    return (18 + 4 * i + s) if i < 2 else (50 + 4 * (i - 2) + s)


def _f32r(x):
    """Round to fp32r (11 explicit mantissa bits, RNE)."""
    b = np.ascontiguousarray(np.asarray(x, np.float32)).view(np.uint32)
    r = (b + np.uint32(0x7FF) + ((b >> np.uint32(12)) & np.uint32(1))) & np.uint32(
        0xFFFFF000
    )
    return r.view(np.float32).copy()


def _build_nc():
    nc = bacc.Bacc("TRN2", target_bir_lowering=False, debug=False,
                   num_devices=N_CORES)

    t0u = nc.dram_tensor("t0u", (NPAIR, 128, CH), F32R, kind="ExternalInput")
    # input-layer weights [KIN, 128] per (eval, row group):
    # rg 0/64 carry the m0-half, rg 32/96 the m1-half
    lin = nc.dram_tensor("lin", (128, N_EVALS * 128), F32R, kind="ExternalInput")
    # final-combine weights [128, 2]: cols (zf, div)
    fin = nc.dram_tensor("fin", (128, 2), F32R, kind="ExternalInput")
    w2 = nc.dram_tensor("w2", (128, 512), BF16, kind="ExternalInput")
    w2gn = nc.dram_tensor("w2gn", (128, 512), BF16, kind="ExternalInput")
    # w3t cols per mo-half: [f, f, df] (f duplicated for the M=2 output)
    w3 = nc.dram_tensor("w3", (128, 6), BF16, kind="ExternalInput")
    c2 = nc.dram_tensor("c2", (128, 2), F32, kind="ExternalInput")
    b2 = nc.dram_tensor("b2", (128, 2), F32, kind="ExternalInput")

    zf = nc.dram_tensor("zf", (NCH, CH), F32R, kind="ExternalOutput")
    dv = nc.dram_tensor("dv", (NCH, CH), F32R, kind="ExternalOutput")

    with tile.TileContext(nc) as tc:
        with (
            tc.tile_pool(name="const", bufs=1) as cpool,
            tc.tile_pool(name="state", bufs=1) as spool,
            tc.tile_pool(name="work", bufs=5) as wpool,
            tc.tile_pool(name="ppre", bufs=1, space="PSUM") as ppre,
            tc.tile_pool(name="pmain", bufs=2, space="PSUM") as pmain,
            tc.tile_pool(name="pcoll", bufs=2, space="PSUM") as pcoll,
        ):
            lint = cpool.tile([128, N_EVALS * 128], F32R)
            fint = cpool.tile([128, 2], F32R)
            w2t = cpool.tile([128, 512], BF16)
            w2gnt = cpool.tile([128, 512], BF16)
            w3t = cpool.tile([128, 6], BF16)
            c2t = cpool.tile([128, 2], F32)
            b2t = cpool.tile([128, 2], F32)
            nc.sync.dma_start(lint[:], lin[:])
            nc.sync.dma_start(fint[:], fin[:])
            nc.sync.dma_start(w2t[:], w2[:])
            nc.sync.dma_start(w2gnt[:], w2gn[:])
            nc.sync.dma_start(w3t[:], w3[:])
            nc.sync.dma_start(c2t[:], c2[:])
            nc.sync.dma_start(b2t[:], b2[:])

            U = []
            for p in range(NPAIR):
                u = spool.tile([128, CH], F32R, tag=f"U{p}")
                nc.sync.dma_start(u[:], t0u[p, :, :])
                U.append(u)

            # round-robin DMA queues for gathers
            dmaq = [nc.sync, nc.gpsimd]
            qi = [0]

            def gdma(dst, src):
                dmaq[qi[0] % 2].dma_start(dst, src)
                qi[0] += 1

            for e in range(N_EVALS):
                i, s = divmod(e, 4)
                krow = 2 + 4 * i + s       # k-row this eval writes
                kdrow = _kd_row(i, s)
                for p in range(NPAIR):
                    Up = U[p]
                    # ---- layer 1: 4 row-tiled K=18 matmuls ----
                    h1b = wpool.tile([128, 2048], BF16, tag="h1")
                    for c in range(2):  # chunk within pair
                        pre = ppre.tile([128, 1024], F32, tag="pre")
                        for m in range(2):  # m-half of hidden1
                            rg = 64 * c + 32 * m
                            nc.tensor.matmul(
                                pre[:, m * CH:(m + 1) * CH],
                                lint[rg:rg + KIN, e * 128:(e + 1) * 128],
                                Up[rg:rg + KIN, :],
                                tile_position=(rg, 0),
                            )
                        nc.scalar.activation(
                            h1b[:, c * 1024:(c + 1) * 1024], pre[:], TANH
                        )
                    sq1 = wpool.tile([128, 2048], BF16, tag="sq1")
                    nc.vector.tensor_tensor(sq1[:], h1b[:], h1b[:], ALU.mult)

                    # ---- layer 2 ----
                    h2 = wpool.tile([128, 2048], BF16, tag="h2")
                    s2 = wpool.tile([128, 2048], BF16, tag="s2")
                    g2 = wpool.tile([128, 2048], BF16, tag="g2")
                    for mo in range(2):
                        mslice = slice(mo * 1024, (mo + 1) * 1024)
                        a2 = pmain.tile([128, 1024], F32, tag="a2")
                        for k in range(2):
                            for c in range(2):
                                nc.tensor.matmul(
                                    a2[:, c * CH:(c + 1) * CH],
                                    w2t[:, k * 256 + mo * 128:
                                        k * 256 + (mo + 1) * 128],
                                    h1b[:, c * 1024 + k * CH:
                                        c * 1024 + (k + 1) * CH],
                                    start=(k == 0),
                                    stop=(k == 1),
                                )
                        nc.scalar.activation(
                            h2[:, mslice], a2[:], TANH,
                            bias=b2t[:, mo:mo + 1],
                        )
                        # s2 = 1 - h2^2 (per mo-half: keeps the g-stream STT
                        # for this half independent of the other half)
                        sq2 = wpool.tile([128, 1024], BF16, tag="sq2")
                        nc.vector.tensor_tensor(
                            sq2[:], h2[:, mslice], h2[:, mslice], ALU.mult
                        )
                        nc.vector.tensor_scalar(
                            s2[:, mslice], sq2[:], -1.0, 1.0,
                            ALU.mult, ALU.add,
                        )
                        g2p = pmain.tile([128, 1024], F32, tag="a2")
                        for k in range(2):
                            for c in range(2):
                                nc.tensor.matmul(
                                    g2p[:, c * CH:(c + 1) * CH],
                                    w2gnt[:, k * 256 + mo * 128:
                                          k * 256 + (mo + 1) * 128],
                                    sq1[:, c * 1024 + k * CH:
                                        c * 1024 + (k + 1) * CH],
                                    start=(k == 0),
                                    stop=(k == 1),
                                )
                        nc.vector.scalar_tensor_tensor(
                            g2[:, mslice], g2p[:],
                            c2t[:, mo:mo + 1],
                            s2[:, mslice],
                            ALU.add, ALU.mult,
                        )

                    # ---- output layer: col-tiled into per-pair coll ----
                    coll = pcoll.tile([128, CH], F32, tag="coll")
                    for mo in range(2):
                        st, sp = (mo == 0), (mo == 1)
                        for c in range(2):
                            # f -> partitions {64c, 64c+1} (M=2 dup)
                            nc.tensor.matmul(
                                coll[64 * c:64 * c + 2, :],
                                w3t[:, 3 * mo:3 * mo + 2],
                                h2[:, mo * 1024 + c * CH:
                                   mo * 1024 + (c + 1) * CH],
                                start=st, stop=sp,
                                tile_position=(0, 64 * c),
                            )
                            # df -> partition {64c+32}
                            nc.tensor.matmul(
                                coll[64 * c + 32:64 * c + 33, :],
                                w3t[:, 3 * mo + 2:3 * mo + 3],
                                g2[:, mo * 1024 + c * CH:
                                   mo * 1024 + (c + 1) * CH],
                                start=st, stop=sp,
                                tile_position=(0, 64 * c + 32),
                            )
                    scr = wpool.tile([128, CH], F32R, tag="scr")
                    nc.scalar.activation(scr[:], coll[:], COPY)

                    # ---- gathers: k/kd rows into U (incl. k replica) ----
                    for c in range(2):
                        base = 64 * c
                        gdma(Up[base + krow:base + krow + 33:32, :],
                             scr[base:base + 2, :])
                        gdma(Up[base + kdrow:base + kdrow + 1, :],
                             scr[base + 32:base + 33, :])

                    if e == N_EVALS - 1:
                        # ---- final combine: zf = z0 + sum(w*k) + b3,
                        # div = sum(w*kd); two accumulating K=26 matmuls ----
                        fca = pmain.tile([128, 1024], F32, tag="a2")
                        fcb = pmain.tile([128, 1024], F32, tag="a2")
                        for c in range(2):
                            base = 64 * c
                            nc.tensor.matmul(
                                fca[0:2, c * CH:(c + 1) * CH],
                                fint[base:base + 26, :],
                                Up[base:base + 26, :],
                                tile_position=(base, 0),
                            )
                            nc.tensor.matmul(
                                fcb[0:2, c * CH:(c + 1) * CH],
                                fint[base + 32:base + 58, :],
                                Up[base + 32:base + 58, :],
                                tile_position=(base + 32, 0),
                            )
                        scrfa = wpool.tile([128, 1024], F32, tag="scrf")
                        nc.scalar.activation(scrfa[0:2, :], fca[0:2, :], COPY)
                        scrf = wpool.tile([128, 1024], F32R, tag="scrf")
                        nc.vector.tensor_tensor(
                            scrf[0:2, :], fcb[0:2, :], scrfa[0:2, :], ALU.add)
                        for c in range(2):
                            ch = 2 * p + c
                            nc.sync.dma_start(
                                zf[ch:ch + 1, :],
                                scrf[0:1, c * CH:(c + 1) * CH])
                            nc.sync.dma_start(
                                dv[ch:ch + 1, :],
                                scrf[1:2, c * CH:(c + 1) * CH])

    nc.compile()
    return nc


_NC_CACHE = None


def _get_nc():
    global _NC_CACHE
    if _NC_CACHE is None:
        _NC_CACHE = _build_nc()
    return _NC_CACHE


def _host_prep(z0, W1, b1, W2, b2, W3, b3):
    """Build per-core input maps (host-side folds; all tiny)."""
    z0 = np.asarray(z0, np.float32)
    W1 = np.asarray(W1, np.float32)
    b1 = np.asarray(b1, np.float32)
    W2 = np.asarray(W2, np.float32)
    b2v = np.asarray(b2, np.float32)
    W3 = np.asarray(W3, np.float32)
    b3v = float(np.asarray(b3, np.float32).reshape(()))

    w1r0, w1r1 = W1[0], W1[1]

    # lin[rg:rg+18, e*128:(e+1)*128]: K=18 input weights; the stage state
    # z_s(e) = z0 + sum_{i'<i} sum_j RK4W[j]*(k_j^{i'} + b3) + c_s*(k_{s-1}^i + b3)
    lin = np.zeros((128, N_EVALS * 128), np.float32)
    for e in range(N_EVALS):
        i, s = divmod(e, 4)
        t_e = i * DT + STAGE_OFF[s]
        c_e = STAGE_C[s]
        blk = np.zeros((KIN, H), np.float32)
        blk[0] = w1r0                                   # z0
        bsum = i * DT + c_e                             # accumulated b3 weight
        blk[1] = t_e * w1r1 + b1 + bsum * b3v * w1r0    # ones row
        for ip in range(i):
            for j in range(4):
                blk[2 + 4 * ip + j] = RK4W[j] * w1r0
        if s >= 1:
            blk[2 + 4 * i + (s - 1)] = c_e * w1r0
        for rg, m in ((0, 0), (32, 1), (64, 0), (96, 1)):
            lin[rg:rg + KIN, e * 128:(e + 1) * 128] = \
                blk[:, m * 128:(m + 1) * 128]

    # final combine weights: rows = U rows; col0 = zf, col1 = div
    fin = np.zeros((128, 2), np.float32)
    for base in (0, 64):
        fin[base + 0, 0] = 1.0                # z0
        fin[base + 1, 0] = b3v                # 4 steps * DT * b3 = b3
        for i in range(N_STEPS):
            for j in range(4):
                fin[base + 2 + 4 * i + j, 0] = RK4W[j]
                fin[base + _kd_row(i, j), 1] = RK4W[j]

    w2p = np.concatenate([W2[0:128, :], W2[128:256, :]], axis=1)  # [128,512]
    w2g = W2 * w1r0[:, None]
    w2gnp = np.concatenate([-w2g[0:128, :], -w2g[128:256, :]], axis=1)
    c2 = w2g.sum(axis=0)  # [256]
    c2p = np.stack([c2[0:128], c2[128:256]], axis=1)  # [128,2]
    b2p = np.stack([b2v[0:128], b2v[128:256]], axis=1)
    # w3t: per mo-half cols [f, f, df]
    w3p = np.zeros((128, 6), np.float32)
    for mo in range(2):
        col = W3[mo * 128:(mo + 1) * 128, 0]
        w3p[:, 3 * mo] = col
        w3p[:, 3 * mo + 1] = col
        w3p[:, 3 * mo + 2] = col

    shared = {
        "lin": _f32r(lin),
        "fin": _f32r(fin),
        "w2": w2p.astype(ml_dtypes.bfloat16),
        "w2gn": w2gnp.astype(ml_dtypes.bfloat16),
        "w3": w3p.astype(ml_dtypes.bfloat16),
        "c2": c2p,
        "b2": b2p,
    }
    in_maps = []
    for core in range(N_CORES):
        zc = z0[core * B:(core + 1) * B, 0].reshape(NCH, CH)
        t0uv = np.zeros((NPAIR, 128, CH), np.float32)
        for p in range(NPAIR):
            for c in range(2):
                base = 64 * c
                zrow = _f32r(zc[2 * p + c])
                t0uv[p, base + 0, :] = zrow
                t0uv[p, base + 1, :] = 1.0
                t0uv[p, base + 32, :] = zrow
                t0uv[p, base + 33, :] = 1.0
        in_maps.append({"t0u": t0uv, **shared})
    return in_maps


def _run(in_maps, **kw):
    nc = _get_nc()
    return run_bass_kernel_spmd(nc, in_maps, core_ids=list(range(N_CORES)), **kw)


def kernel(z0, W1, b1, W2, b2, W3, b3):
    in_maps = _host_prep(z0, W1, b1, W2, b2, W3, b3)
    res = _run(in_maps)
    zf = np.concatenate(
        [np.asarray(r["zf"], np.float32).reshape(B, 1) for r in res.results]
    )
    dv = np.concatenate(
        [np.asarray(r["dv"], np.float32).reshape(B, 1) for r in res.results]
    )
    return zf, dv


# revision 21
# speedup vs baseline: 1.9216x; 1.0531x over previous
"""Trainium2 Bass kernel for nn_CNF1D: 1-D continuous normalizing flow.

Reference computation (per sample b, D=1, H=256, RK4 with 4 steps over [0,1]):
    f(t,z):  h1 = tanh(z*W1[0] + t*W1[1] + b1); h2 = tanh(h1@W2 + b2);
             f = h2@W3 + b3
    JVP:     s1 = 1-h1^2;  g2 = (1-h2^2) * ((s1*W1[0])@W2);  df = g2@W3
    (z, div) integrated with RK4; outputs (z_final, div_integral).

Strategy: pure data parallelism over 8 cores (4096 samples each), 8 chunks
of 512 samples per core, processed as 4 chunk-pairs sharing an output
collector. Hidden-major layout ([hidden, batch]).

K-HISTORY TRICK - no per-step RK4 combine: every stage state z_s is LINEAR
in (z0, k-history), so the K=18 input matmul reconstructs z_s directly from
the stored k-rows with host-folded weights. k/kd values accumulate in the
per-chunk U tile across all 16 evals; one final matmul pair per chunk
produces (zf, div).

Per-chunk state tile U [128, 512] (fp32r):
  0: z0   1: ones   2..17: k1..k4 steps 0..3   18..25: kd steps 0,1
  32..49: replica of 0..17 (feeds the 2nd row-tiled input matmul)
  50..57: kd steps 2,3
Input layer: 2 concurrent row-tiled K=18 matmuls (row groups 0 and 32).
Output layer: col-tiled bf16 matmuls into a shared per-pair collector;
f duplicated to 2 partitions (M=2) so the replica k-row gets its own source.
One combined k+kd gather DMA per chunk (strided partitions) + one replica
gather.

dtypes: state + input/final matmuls fp32r; streams bf16; fp32 PSUM;
tanh on ScalarE from PSUM; JVP elementwise on VectorE.
"""

import sys

for _p in ("/opt/trn_rl_repo",):
    if _p not in sys.path:
        sys.path.insert(0, _p)

import numpy as np
import ml_dtypes

import concourse.mybir as mybir
from concourse import bacc, tile
from concourse.bass_utils import run_bass_kernel_spmd

F32 = mybir.dt.float32
F32R = mybir.dt.float32r
BF16 = mybir.dt.bfloat16
ALU = mybir.AluOpType
TANH = mybir.ActivationFunctionType.Tanh
COPY = mybir.ActivationFunctionType.Copy

N_CORES = 8
B_TOT = 32768
B = B_TOT // N_CORES        # 4096 per core
H = 256                     # hidden
CH = 512                    # chunk (matmul N / psum bank)
NCH = B // CH               # 8 chunks per core
N_STEPS = 4
DT = 1.0 / N_STEPS
N_EVALS = 4 * N_STEPS       # 16
STAGE_OFF = [0.0, DT / 2, DT / 2, DT]
STAGE_C = [0.0, DT / 2, DT / 2, DT]
RK4W = [DT / 6, DT / 3, DT / 3, DT / 6]
KIN = 18                    # input contraction rows: z0, ones, 16 k-rows


def _kd_row(i, s):
    """U row of kd for step i, stage s."""
    return (18 + 4 * i + s) if i < 2 else (50 + 4 * (i - 2) + s)


def _f32r(x):
    """Round to fp32r (11 explicit mantissa bits, RNE)."""
    b = np.ascontiguousarray(np.asarray(x, np.float32)).view(np.uint32)
    r = (b + np.uint32(0x7FF) + ((b >> np.uint32(12)) & np.uint32(1))) & np.uint32(
        0xFFFFF000
    )
    return r.view(np.float32).copy()


def _build_nc():
    nc = bacc.Bacc("TRN2", target_bir_lowering=False, debug=False,
                   num_devices=N_CORES)

    t0u = nc.dram_tensor("t0u", (NCH, 128, CH), F32R, kind="ExternalInput")
    # input weights: rows 0-17 = m0-half block, rows 32-49 = m1-half block
    lin = nc.dram_tensor("lin", (128, N_EVALS * 128), F32R, kind="ExternalInput")
    # final-combine weights [128, 2]: cols (zf, div); row blocks 0-25 / 32-57
    fin = nc.dram_tensor("fin", (128, 2), F32R, kind="ExternalInput")
    w2 = nc.dram_tensor("w2", (128, 512), BF16, kind="ExternalInput")
    w2gn = nc.dram_tensor("w2gn", (128, 512), BF16, kind="ExternalInput")
    # w3t cols per k-half: [f, f, df]
    w3 = nc.dram_tensor("w3", (128, 6), BF16, kind="ExternalInput")
    c2 = nc.dram_tensor("c2", (128, 2), F32, kind="ExternalInput")
    b2 = nc.dram_tensor("b2", (128, 2), F32, kind="ExternalInput")

    zf = nc.dram_tensor("zf", (NCH, CH), F32R, kind="ExternalOutput")
    dv = nc.dram_tensor("dv", (NCH, CH), F32R, kind="ExternalOutput")

    with tile.TileContext(nc) as tc:
        with (
            tc.tile_pool(name="const", bufs=1) as cpool,
            tc.tile_pool(name="state", bufs=1) as spool,
            tc.tile_pool(name="work", bufs=12) as wpool,
            tc.tile_pool(name="psum", bufs=2, space="PSUM") as ppool,
        ):
            lint = cpool.tile([128, N_EVALS * 128], F32R)
            fint = cpool.tile([128, 2], F32R)
            w2t = cpool.tile([128, 512], BF16)
            w2gnt = cpool.tile([128, 512], BF16)
            w3t = cpool.tile([128, 6], BF16)
            c2t = cpool.tile([128, 2], F32)
            b2t = cpool.tile([128, 2], F32)
            nc.sync.dma_start(lint[:], lin[:])
            nc.sync.dma_start(fint[:], fin[:])
            nc.sync.dma_start(w2t[:], w2[:])
            nc.sync.dma_start(w2gnt[:], w2gn[:])
            nc.sync.dma_start(w3t[:], w3[:])
            nc.sync.dma_start(c2t[:], c2[:])
            nc.sync.dma_start(b2t[:], b2[:])

            U = []
            for c in range(NCH):
                u = spool.tile([128, CH], F32R, tag=f"U{c}")
                nc.sync.dma_start(u[:], t0u[c, :, :])
                U.append(u)

            for e in range(N_EVALS):
                i, s = divmod(e, 4)
                krow = 2 + 4 * i + s
                kdrow = _kd_row(i, s)
                dk = kdrow - krow          # 16 (steps 0,1) or 40 (steps 2,3)
                for cp in range(NCH // 2):
                    pair_h2g2 = []
                    for ci in range(2):
                        c = 2 * cp + ci
                        Uc = U[c]
                        # input layer: 2 row-tiled K=18 matmuls
                        h1 = wpool.tile([128, 2 * CH], BF16, tag="h1")
                        for m in range(2):
                            rg = 32 * m
                            pre1 = ppool.tile([128, CH], F32, tag="pre1")
                            nc.tensor.matmul(
                                pre1[:],
                                lint[rg:rg + KIN, e * 128:(e + 1) * 128],
                                Uc[rg:rg + KIN, :],
                                tile_position=(rg, 0),
                            )
                            nc.scalar.activation(
                                h1[:, m * CH:(m + 1) * CH], pre1[:], TANH
                            )
                        sq1 = wpool.tile([128, 2 * CH], BF16, tag="sq1")
                        nc.vector.tensor_tensor(sq1[:], h1[:], h1[:], ALU.mult)
                        # layer 2: h-stream (W2) and g-stream (-W2g, rhs=h1^2)
                        h2 = wpool.tile([128, 2 * CH], BF16, tag="h2")
                        g2ps = []
                        for mo in range(2):
                            a2 = ppool.tile([128, CH], F32, tag="a2")
                            for k in range(2):
                                nc.tensor.matmul(
                                    a2[:],
                                    w2t[:, k * 256 + mo * 128:k * 256 + (mo + 1) * 128],
                                    h1[:, k * CH:(k + 1) * CH],
                                    start=(k == 0),
                                    stop=(k == 1),
                                )
                            nc.scalar.activation(
                                h2[:, mo * CH:(mo + 1) * CH], a2[:], TANH,
                                bias=b2t[:, mo:mo + 1],
                            )
                            g2p = ppool.tile([128, CH], F32, tag="g2p")
                            for k in range(2):
                                nc.tensor.matmul(
                                    g2p[:],
                                    w2gnt[:, k * 256 + mo * 128:k * 256 + (mo + 1) * 128],
                                    sq1[:, k * CH:(k + 1) * CH],
                                    start=(k == 0),
                                    stop=(k == 1),
                                )
                            g2ps.append(g2p)
                        sq2 = wpool.tile([128, 2 * CH], BF16, tag="sq2")
                        nc.vector.tensor_tensor(sq2[:], h2[:], h2[:], ALU.mult)
                        s2 = wpool.tile([128, 2 * CH], BF16, tag="s2")
                        nc.vector.tensor_scalar(s2[:], sq2[:], -1.0, 1.0, ALU.mult, ALU.add)
                        g2 = wpool.tile([128, 2 * CH], BF16, tag="g2")
                        for mo in range(2):
                            # g2 = (g2p + C2) * (1 - h2^2)
                            nc.vector.scalar_tensor_tensor(
                                g2[:, mo * CH:(mo + 1) * CH], g2ps[mo][:],
                                c2t[:, mo:mo + 1], s2[:, mo * CH:(mo + 1) * CH],
                                ALU.add, ALU.mult,
                            )
                        pair_h2g2.append((h2, g2))
                    # output layer for BOTH chunks into one collector:
                    # chunk ci: f (M=2 dup) -> partitions {64ci, 64ci+1},
                    #           df -> partition 64ci+32
                    coll = ppool.tile([128, CH], F32, tag="coll")
                    for k in range(2):
                        for ci in range(2):
                            h2, g2 = pair_h2g2[ci]
                            pf = 64 * ci
                            nc.tensor.matmul(
                                coll[pf:pf + 2, :], w3t[:, 3 * k:3 * k + 2],
                                h2[:, k * CH:(k + 1) * CH],
                                start=(k == 0), stop=(k == 1),
                                tile_position=(0, pf),
                            )
                            nc.tensor.matmul(
                                coll[pf + 32:pf + 33, :], w3t[:, 3 * k + 2:3 * k + 3],
                                g2[:, k * CH:(k + 1) * CH],
                                start=(k == 0), stop=(k == 1),
                                tile_position=(0, pf + 32),
                            )
                    scr = wpool.tile([128, CH], F32R, tag="scr")
                    nc.scalar.activation(scr[:], coll[:], COPY)
                    for ci in range(2):
                        c = 2 * cp + ci
                        base = 64 * ci
                        dma_eng = nc.sync if ci == 0 else nc.gpsimd
                        # k -> row krow, kd -> row krow+dk (one strided DMA)
                        dma_eng.dma_start(
                            U[c][krow:krow + dk + 1:dk, :],
                            scr[base:base + 33:32, :],
                        )
                        # f replica -> row 32+krow
                        dma_eng.dma_start(
                            U[c][32 + krow:33 + krow, :],
                            scr[base + 1:base + 2, :],
                        )
                    if e == N_EVALS - 1:
                        # final combine per chunk: two K=26 matmuls
                        # (fp32r forbids col-tiling; separate tiles + V-add)
                        for ci in range(2):
                            c = 2 * cp + ci
                            fca = ppool.tile([128, CH], F32, tag="a2")
                            fcb = ppool.tile([128, CH], F32, tag="g2p")
                            nc.tensor.matmul(
                                fca[0:2, :],
                                fint[0:26, :], U[c][0:26, :],
                                tile_position=(0, 0),
                            )
                            nc.tensor.matmul(
                                fcb[0:2, :],
                                fint[32:58, :], U[c][32:58, :],
                                tile_position=(32, 0),
                            )
                            scrfa = wpool.tile([128, CH], F32, tag="scr")
                            nc.scalar.activation(
                                scrfa[0:2, :], fca[0:2, :], COPY)
                            scrf = wpool.tile([128, CH], F32R, tag="scr")
                            nc.vector.tensor_tensor(
                                scrf[0:2, :], fcb[0:2, :], scrfa[0:2, :],
                                ALU.add)
                            nc.sync.dma_start(zf[c:c + 1, :], scrf[0:1, :])
                            nc.sync.dma_start(dv[c:c + 1, :], scrf[1:2, :])

    nc.compile()
    return nc


_NC_CACHE = None


def _get_nc():
    global _NC_CACHE
    if _NC_CACHE is None:
        _NC_CACHE = _build_nc()
    return _NC_CACHE


def _host_prep(z0, W1, b1, W2, b2, W3, b3):
    """Build per-core input maps (host-side folds; all tiny)."""
    z0 = np.asarray(z0, np.float32)
    W1 = np.asarray(W1, np.float32)
    b1 = np.asarray(b1, np.float32)
    W2 = np.asarray(W2, np.float32)
    b2v = np.asarray(b2, np.float32)
    W3 = np.asarray(W3, np.float32)
    b3v = float(np.asarray(b3, np.float32).reshape(()))

    w1r0, w1r1 = W1[0], W1[1]

    # K=18 input weights; z_s(e) = z0 + sum_{i'<i} sum_j RK4W[j]*(k+b3)
    #                              + c_s*(k_{s-1}+b3)
    lin = np.zeros((128, N_EVALS * 128), np.float32)
    for e in range(N_EVALS):
        i, s = divmod(e, 4)
        t_e = i * DT + STAGE_OFF[s]
        c_e = STAGE_C[s]
        blk = np.zeros((KIN, H), np.float32)
        blk[0] = w1r0
        bsum = i * DT + c_e
        blk[1] = t_e * w1r1 + b1 + bsum * b3v * w1r0
        for ip in range(i):
            for j in range(4):
                blk[2 + 4 * ip + j] = RK4W[j] * w1r0
        if s >= 1:
            blk[2 + 4 * i + (s - 1)] = c_e * w1r0
        lin[0:KIN, e * 128:(e + 1) * 128] = blk[:, 0:128]
        lin[32:32 + KIN, e * 128:(e + 1) * 128] = blk[:, 128:256]

    # final combine weights: col0 = zf, col1 = div
    fin = np.zeros((128, 2), np.float32)
    fin[0, 0] = 1.0
    fin[1, 0] = b3v                # 4 steps * DT * b3
    for i in range(N_STEPS):
        for j in range(4):
            fin[2 + 4 * i + j, 0] = RK4W[j]
            fin[_kd_row(i, j), 1] = RK4W[j]

    w2p = np.concatenate([W2[0:128, :], W2[128:256, :]], axis=1)  # [128,512]
    w2g = W2 * w1r0[:, None]
    w2gnp = np.concatenate([-w2g[0:128, :], -w2g[128:256, :]], axis=1)
    c2 = w2g.sum(axis=0)  # [256]
    c2p = np.stack([c2[0:128], c2[128:256]], axis=1)  # [128,2]
    b2p = np.stack([b2v[0:128], b2v[128:256]], axis=1)
    # w3t: per k-half cols [f, f, df]
    w3p = np.zeros((128, 6), np.float32)
    for k in range(2):
        col = W3[k * 128:(k + 1) * 128, 0]
        w3p[:, 3 * k] = col
        w3p[:, 3 * k + 1] = col
        w3p[:, 3 * k + 2] = col

    shared = {
        "lin": _f32r(lin),
        "fin": _f32r(fin),
        "w2": w2p.astype(ml_dtypes.bfloat16),
        "w2gn": w2gnp.astype(ml_dtypes.bfloat16),
        "w3": w3p.astype(ml_dtypes.bfloat16),
        "c2": c2p,
        "b2": b2p,
    }
    in_maps = []
    for core in range(N_CORES):
        zc = z0[core * B:(core + 1) * B, 0].reshape(NCH, CH)
        t0uv = np.zeros((NCH, 128, CH), np.float32)
        for c in range(NCH):
            zrow = _f32r(zc[c])
            t0uv[c, 0, :] = zrow
            t0uv[c, 1, :] = 1.0
            t0uv[c, 32, :] = zrow
            t0uv[c, 33, :] = 1.0
        in_maps.append({"t0u": t0uv, **shared})
    return in_maps


def _run(in_maps, **kw):
    nc = _get_nc()
    return run_bass_kernel_spmd(nc, in_maps, core_ids=list(range(N_CORES)), **kw)


def kernel(z0, W1, b1, W2, b2, W3, b3):
    in_maps = _host_prep(z0, W1, b1, W2, b2, W3, b3)
    res = _run(in_maps)
    zf = np.concatenate(
        [np.asarray(r["zf"], np.float32).reshape(B, 1) for r in res.results]
    )
    dv = np.concatenate(
        [np.asarray(r["dv"], np.float32).reshape(B, 1) for r in res.results]
    )
    return zf, dv
